# revision 18
# baseline (speedup 1.0000x reference)
"""Trainium2 Bass kernel for nn_Atten_Block (non-local attention block).

Reference computation per batch element b (C=256, C4=64, H=W=64, N=4096):
    theta = W1 @ x + b1          [C4, N]
    phi   = W2 @ x + b2          [C4, N]
    g     = W3 @ x + b3          [C4, N]
    S     = theta^T @ phi        [N, N]
    A     = softmax(S, axis=-1)
    attn_g[c,i] = sum_j g[c,j] A[i,j]
    y     = x + W4 @ attn_g + b4

Sharding: data-parallel over batch B=8 across the 8 NeuronCores (one batch
element per core).

Per-core algorithm (engine-balanced around the ScalarE exp bottleneck):
  - S is computed TRANSPOSED: S^T tile [j=128, i=512] = phi_jblk.T @ theta_i
    so that softmax normalization and the PV matmul need no transposes:
      P^T = exp(S^T)  (no max-subtraction: |S| <= ~65 < 88, safe in fp32)
      pv[c,i] = sum_j gT[j,c] P^T[j,i]  via matmul with lhsT = [gT | ones]
    The appended ones column makes pv row 64 the softmax denominators l[i].
  - attn_g = pv[0:64] * (1/l) broadcast via a K=1 ones matmul.
  - y = x + W4 @ attn_g + b4 fused in one DVE op per tile.

Matmuls run in float32r (1 cyc/row vs fp32's 4) — producers round to f32r.
"""

import sys
from contextlib import ExitStack

import numpy as np

if "/opt/trn_rl_repo" not in sys.path:
    sys.path.insert(0, "/opt/trn_rl_repo")

C = 256
C4 = 64
B = 8
H = W = 64
N = H * W          # 4096
NI = 512           # i-tile width (matmul free dim)
NJ = 128           # j-block (S^T partition dim)
N_ITILES = N // NI   # 8
N_JBLKS = N // NJ    # 32

_CACHE = {}


def _build(cfg):
    import concourse.tile as tile
    from concourse import bacc, mybir

    F32 = mybir.dt.float32

    nc = bacc.Bacc("TRN2", target_bir_lowering=False, debug=False,
                   num_devices=B)

    aps = dict(
        x_d=nc.dram_tensor("x", [128, 2 * N], F32, kind="ExternalInput").ap(),
        w1_d=nc.dram_tensor("w1t", [128, 256], F32, kind="ExternalInput").ap(),
        w2_d=nc.dram_tensor("w2t", [128, 256], F32, kind="ExternalInput").ap(),
        w3_d=nc.dram_tensor("w3t", [128, 128], F32, kind="ExternalInput").ap(),
        w4_d=nc.dram_tensor("w4t", [C4, C], F32, kind="ExternalInput").ap(),
        b123_d=nc.dram_tensor("b123", [128, 3], F32, kind="ExternalInput").ap(),
        b4_d=nc.dram_tensor("b4c", [128, 2], F32, kind="ExternalInput").ap(),
        y_d=nc.dram_tensor("y", [C, N], F32, kind="ExternalOutput").ap(),
    )

    with tile.TileContext(nc) as tc:
        _body(nc, tc, cfg, aps)
    nc.compile()
    return nc


def _body(nc, tc, cfg, aps):
    import concourse.tile as tile  # noqa: F401
    from concourse import masks, mybir
    from concourse.alu_op_type import AluOpType as Alu

    F32 = mybir.dt.float32
    F32R = mybir.dt.float32r
    MM = F32R if cfg.get("f32r", True) else F32
    Exp = mybir.ActivationFunctionType.Exp

    x_d, y_d = aps["x_d"], aps["y_d"]

    with ExitStack() as st:
        sb = st.enter_context(tc.tile_pool(name="sb", bufs=1))

        # ---- static SBUF tensors ----
        x_sb = sb.tile([128, 2 * N], F32, tag="x_sb")      # residual source
        xr_sb = sb.tile([128, 2 * N], MM, tag="xr_sb")     # rounded for matmul
        # theta/phi duplicated across both partition halves (rows 64-127 =
        # rows 0-63) so S^T matmul pairs can row-pack the full PE array.
        th_sb = sb.tile([128, N], MM, tag="th_sb")         # theta (dup)
        ph_sb = sb.tile([128, N], MM, tag="ph_sb")         # phi (dup)
        g_sb = sb.tile([C4, N], F32, tag="g_sb")           # g (pre-transpose)
        gt_sb = sb.tile([128, N_JBLKS * (C4 + 1)], MM, tag="gt_sb")  # [gT|1]
        w1_sb = sb.tile([128, 256], MM, tag="w1_sb")       # dup-M k-tiles
        w2_sb = sb.tile([128, 256], MM, tag="w2_sb")
        w3_sb = sb.tile([128, 128], MM, tag="w3_sb")
        w4_sb = sb.tile([C4, C], MM, tag="w4_sb")
        wtmp_sb = sb.tile([128, C], F32, tag="wtmp_sb")
        b123_sb = sb.tile([128, 3], F32, tag="b123_sb")
        b4_sb = sb.tile([128, 2], F32, tag="b4_sb")
        eye_sb = sb.tile([C4, C4], F32, tag="eye_sb")
        ones_sb = sb.tile([1, C4], F32, tag="ones_sb")

        # weights in (rounded to f32r via DVE copy)
        for w_d, w_sb in ((aps["w1_d"], w1_sb), (aps["w2_d"], w2_sb)):
            nc.sync.dma_start(wtmp_sb[:], w_d[:])
            nc.vector.tensor_copy(w_sb[:], wtmp_sb[:])
        nc.sync.dma_start(wtmp_sb[:, 0:128], aps["w3_d"][:])
        nc.vector.tensor_copy(w3_sb[:], wtmp_sb[:, 0:128])
        nc.sync.dma_start(wtmp_sb[0:C4, 0:C], aps["w4_d"][:])
        nc.vector.tensor_copy(w4_sb[:], wtmp_sb[0:C4, 0:C])
        nc.sync.dma_start(b123_sb[:], aps["b123_d"][:])
        nc.sync.dma_start(b4_sb[:], aps["b4_d"][:])
        masks.make_identity(nc, eye_sb[:])
        nc.vector.memset(ones_sb[:], 1.0)
        ones_col = sb.tile([128, N_JBLKS], F32, tag="ones_col")
        nc.vector.memset(ones_col[:], 1.0)
        nc.vector.tensor_copy(
            gt_sb[:].rearrange("p (j c) -> p j c", c=C4 + 1)
            [:, :, C4:C4 + 1],
            ones_col[:].rearrange("p (j c) -> p j c", c=1))

        # x in, chunked; round-copy to f32r
        NCH = 1024
        for c0 in range(0, 2 * N, NCH):
            nc.sync.dma_start(x_sb[:, c0:c0 + NCH], x_d[:, c0:c0 + NCH])
            nc.vector.tensor_copy(xr_sb[:, c0:c0 + NCH], x_sb[:, c0:c0 + NCH])

        # ---- phase A: theta / phi / g conv1x1; gT via PE transpose ----
        with tc.tile_pool(name="psA", bufs=2, space="PSUM") as psA:

            def conv(dst_sb, w_sb_, bias_col, m):
                for n in range(N_ITILES):
                    ps = psA.tile([128, NI], F32, tag="convps")
                    for k in range(2):
                        nc.tensor.matmul(
                            ps[0:m, :],
                            w_sb_[:, k * m:(k + 1) * m],
                            xr_sb[:, k * N + n * NI:k * N + (n + 1) * NI],
                            start=(k == 0), stop=(k == 1))
                    # bias-add + PSUM->SBUF (+ rounding) in one DVE op
                    nc.vector.tensor_scalar_add(
                        dst_sb[0:m, n * NI:(n + 1) * NI], ps[0:m, :],
                        b123_sb[0:m, bias_col:bias_col + 1])

            conv(ph_sb, w2_sb, 1, 128)   # phi first: S^T needs all of phi
            conv(th_sb, w1_sb, 0, 128)
            conv(g_sb, w3_sb, 2, C4)

            # gT: transpose g 128-col blocks -> [128, 64] each
            for j in range(N_JBLKS):
                tp = psA.tile([128, C4], F32, tag="tpps")
                nc.tensor.transpose(tp[:], g_sb[:, j * NJ:(j + 1) * NJ],
                                    eye_sb[:])
                nc.vector.tensor_copy(
                    gt_sb[:, j * (C4 + 1):j * (C4 + 1) + C4], tp[:])

        # ---- main loop ----
        SBANKS = cfg.get("stage_banks", 4)      # psum banks for S^T staging
        HALF = SBANKS // 2 * 512                # cols per staging half
        JPB = HALF // NI                        # j-blocks per exp batch
        ps_stage = st.enter_context(
            tc.tile_pool(name="ps_stage", bufs=2, space="PSUM"))
        ps_pv = st.enter_context(
            tc.tile_pool(name="ps_pv", bufs=2, space="PSUM"))
        ps_misc = st.enter_context(
            tc.tile_pool(name="ps_misc", bufs=1, space="PSUM"))
        pt_pool = st.enter_context(tc.tile_pool(name="pt", bufs=2))
        dv_pool = st.enter_context(tc.tile_pool(name="dv", bufs=2))
        y_pool = st.enter_context(tc.tile_pool(name="yp", bufs=3))

        batches = []
        j = 0
        while j < N_JBLKS:
            nb = min(JPB, N_JBLKS - j)
            batches.append(list(range(j, j + nb)))
            j += nb
        NB = len(batches)

        pvs = [None] * N_ITILES

        def emit_s(i, b):
            # row-packed pairs: even j-blocks on PE rows 0-63, odd on 64-127
            # (theta/phi are duplicated across halves) -> concurrent matmuls
            # and full-array activity for the HAM clock gate.
            stage_t = ps_stage.tile([128, HALF], F32, tag="stage",
                                    name=f"stage_{i}_{b}")
            half = stage_t[:, 0:len(batches[b]) * NI]
            for k, jb in enumerate(batches[b]):
                lo = (k % 2) * C4
                nc.tensor.matmul(
                    half[:, k * NI:(k + 1) * NI],
                    ph_sb[lo:lo + C4, jb * NJ:(jb + 1) * NJ],
                    th_sb[lo:lo + C4, i * NI:(i + 1) * NI],
                    start=True, stop=True)
            return half

        def emit_exp_pv(i, b, half):
            w = len(batches[b]) * NI
            pt = pt_pool.tile([128, HALF], MM, tag="pt")
            nc.scalar.activation(pt[:, 0:w], half[:], Exp)
            pv = pvs[i]
            for k, jb in enumerate(batches[b]):
                nc.tensor.matmul(
                    pv[0:C4 + 1, :],
                    gt_sb[:, jb * (C4 + 1):(jb + 1) * (C4 + 1)],
                    pt[:, k * NI:(k + 1) * NI],
                    start=(jb == 0), stop=(jb == N_JBLKS - 1))

        def emit_tail(i):
            pv = pvs[i]
            recip = dv_pool.tile([1, NI], F32, tag="recip")
            nc.vector.reciprocal(recip[:], pv[C4:C4 + 1, :])
            bc = ps_misc.tile([128, NI], F32, tag="misc")
            nc.tensor.matmul(bc[0:C4, :], ones_sb[:], recip[:],
                             start=True, stop=True)
            bcs = dv_pool.tile([C4, NI], F32, tag="bcs")
            nc.vector.tensor_copy(bcs[:], bc[0:C4, :])
            ag = dv_pool.tile([C4, NI], MM, tag="ag")
            nc.vector.tensor_tensor(ag[:], pv[0:C4, :], bcs[:], Alu.mult)
            for h in range(2):
                z = ps_misc.tile([128, NI], F32, tag="misc")
                nc.tensor.matmul(z[:], w4_sb[:, h * 128:(h + 1) * 128],
                                 ag[:], start=True, stop=True)
                yt = y_pool.tile([128, NI], F32, tag="yt")
                # y = (z + b4) + x
                nc.vector.scalar_tensor_tensor(
                    yt[:], z[:], b4_sb[:, h:h + 1],
                    x_sb[:, h * N + i * NI:h * N + (i + 1) * NI],
                    Alu.add, Alu.add)
                nc.sync.dma_start(
                    y_d[h * 128:(h + 1) * 128, i * NI:(i + 1) * NI], yt[:])

        # software-pipelined emission: S(b+1) lands before exp/PV(b) on the
        # PE stream; the previous i-tile's tail is deferred past the first S
        # batch of the next i-tile so ACT never waits on the tail chain.
        pending_tail = None
        TAIL_AT = 3   # defer prev tail this many batches into the next i-tile
        for i in range(N_ITILES):
            pvs[i] = ps_pv.tile([128, NI], F32, tag="pv", name=f"pv{i}")
            halves = [None] * NB
            halves[0] = emit_s(i, 0)
            for b in range(NB):
                if b + 1 < NB:
                    halves[b + 1] = emit_s(i, b + 1)
                emit_exp_pv(i, b, halves[b])
                if b == TAIL_AT and pending_tail is not None:
                    emit_tail(pending_tail)
                    pending_tail = None
            pending_tail = i
        emit_tail(pending_tail)


def _prepare_core_inputs(x_b, W1, b1, W2, b2, W3, b3, W4, b4):
    def ktile(wT, m):
        # [256, m] -> [128, 2*m] (two k-tiles side by side)
        return np.ascontiguousarray(
            wT.reshape(2, 128, m).transpose(1, 0, 2).reshape(128, 2 * m))

    def dup(wT):
        # duplicate output channels across both halves: [256,64] -> [256,128]
        return np.concatenate([wT, wT], axis=1)

    z64 = np.zeros(C4, np.float32)
    return {
        "x": np.ascontiguousarray(
            x_b.reshape(2, 128, N).transpose(1, 0, 2).reshape(128, 2 * N)),
        "w1t": ktile(dup(W1.T), 128), "w2t": ktile(dup(W2.T), 128),
        "w3t": ktile(W3.T, C4),
        "w4t": np.ascontiguousarray(W4.T),
        "b123": np.ascontiguousarray(
            np.stack([np.r_[b1, b1], np.r_[b2, b2], np.r_[b3, z64]], axis=1)),
        "b4c": np.ascontiguousarray(b4.reshape(2, 128).T),
    }


def kernel(x, W1, b1, W2, b2, W3, b3, W4, b4, _trace=False, _cfg=None):
    from concourse import bass_utils

    cfg = dict(_cfg or {})
    key = tuple(sorted(cfg.items()))
    if key not in _CACHE:
        _CACHE[key] = _build(cfg)
    nc = _CACHE[key]

    x = np.asarray(x, dtype=np.float32)
    xf = x.reshape(B, C, N)
    args = [np.asarray(a, dtype=np.float32)
            for a in (W1, b1, W2, b2, W3, b3, W4, b4)]
    in_maps = [_prepare_core_inputs(xf[b], *args) for b in range(B)]
    res = bass_utils.run_bass_kernel_spmd(
        nc, in_maps, core_ids=list(range(B)), trace=_trace)
    out = np.stack([res.results[b]["y"].reshape(C, H, W) for b in range(B)])
    if _trace:
        kernel.last_exec_time_ns = res.exec_time_ns
    return out


# revision 19
# speedup vs baseline: 1.0399x; 1.0399x over previous
"""Trainium2 Bass kernel for nn_Atten_Block (non-local attention block).

Reference computation per batch element b (C=256, C4=64, H=W=64, N=4096):
    theta = W1 @ x + b1          [C4, N]
    phi   = W2 @ x + b2          [C4, N]
    g     = W3 @ x + b3          [C4, N]
    S     = theta^T @ phi        [N, N]
    A     = softmax(S, axis=-1)
    attn_g[c,i] = sum_j g[c,j] A[i,j]
    y     = x + W4 @ attn_g + b4

Sharding: data-parallel over batch B=8 across the 8 NeuronCores (one batch
element per core).

Per-core algorithm (engine-balanced around the ScalarE exp bottleneck):
  - S is computed TRANSPOSED: S^T tile [j=128, i=512] = phi_jblk.T @ theta_i
    so that softmax normalization and the PV matmul need no transposes:
      P^T = exp(S^T)  (no max-subtraction: |S| <= ~65 < 88, safe in fp32)
      pv[c,i] = sum_j gT[j,c] P^T[j,i]  via matmul with lhsT = [gT | ones]
    The appended ones column makes pv row 64 the softmax denominators l[i].
  - attn_g = pv[0:64] * (1/l) broadcast via a K=1 ones matmul.
  - y = x + W4 @ attn_g + b4 fused in one DVE op per tile.

Matmuls run in float32r (1 cyc/row vs fp32's 4) — producers round to f32r.
"""

import sys
from contextlib import ExitStack

import numpy as np

if "/opt/trn_rl_repo" not in sys.path:
    sys.path.insert(0, "/opt/trn_rl_repo")

C = 256
C4 = 64
B = 8
H = W = 64
N = H * W          # 4096
NI = 512           # i-tile width (matmul free dim)
NJ = 128           # j-block (S^T partition dim)
N_ITILES = N // NI   # 8
N_JBLKS = N // NJ    # 32

_CACHE = {}


def _build(cfg):
    import concourse.tile as tile
    from concourse import bacc, mybir

    F32 = mybir.dt.float32

    nc = bacc.Bacc("TRN2", target_bir_lowering=False, debug=False,
                   num_devices=B)

    aps = dict(
        x_d=nc.dram_tensor("x", [128, 2 * N], F32, kind="ExternalInput").ap(),
        w1_d=nc.dram_tensor("w1t", [128, 256], F32, kind="ExternalInput").ap(),
        w2_d=nc.dram_tensor("w2t", [128, 256], F32, kind="ExternalInput").ap(),
        w3_d=nc.dram_tensor("w3t", [128, 128], F32, kind="ExternalInput").ap(),
        w4_d=nc.dram_tensor("w4t", [C4, C], F32, kind="ExternalInput").ap(),
        b123_d=nc.dram_tensor("b123", [128, 3], F32, kind="ExternalInput").ap(),
        b4_d=nc.dram_tensor("b4c", [128, 2], F32, kind="ExternalInput").ap(),
        y_d=nc.dram_tensor("y", [C, N], F32, kind="ExternalOutput").ap(),
    )

    with tile.TileContext(nc) as tc:
        _body(nc, tc, cfg, aps)
    nc.compile()
    return nc


def _body(nc, tc, cfg, aps):
    import concourse.tile as tile  # noqa: F401
    from concourse import masks, mybir
    from concourse.alu_op_type import AluOpType as Alu

    F32 = mybir.dt.float32
    F32R = mybir.dt.float32r
    MM = F32R if cfg.get("f32r", True) else F32
    Exp = mybir.ActivationFunctionType.Exp

    x_d, y_d = aps["x_d"], aps["y_d"]

    with ExitStack() as st:
        sb = st.enter_context(tc.tile_pool(name="sb", bufs=1))

        # ---- static SBUF tensors ----
        x_sb = sb.tile([128, 2 * N], F32, tag="x_sb")      # residual source
        xr_sb = sb.tile([128, 2 * N], MM, tag="xr_sb")     # rounded for matmul
        # theta/phi duplicated across both partition halves (rows 64-127 =
        # rows 0-63) so S^T matmul pairs can row-pack the full PE array.
        th_sb = sb.tile([128, N], MM, tag="th_sb")         # theta (dup)
        ph_sb = sb.tile([128, N], MM, tag="ph_sb")         # phi (dup)
        g_sb = sb.tile([C4, N], F32, tag="g_sb")           # g (pre-transpose)
        gt_sb = sb.tile([128, N_JBLKS * (C4 + 1)], MM, tag="gt_sb")  # [gT|1]
        w1_sb = sb.tile([128, 256], MM, tag="w1_sb")       # dup-M k-tiles
        w2_sb = sb.tile([128, 256], MM, tag="w2_sb")
        w3_sb = sb.tile([128, 128], MM, tag="w3_sb")
        w4_sb = sb.tile([C4, C], MM, tag="w4_sb")
        wtmp_sb = sb.tile([128, C], F32, tag="wtmp_sb")
        b123_sb = sb.tile([128, 3], F32, tag="b123_sb")
        b4_sb = sb.tile([128, 2], F32, tag="b4_sb")
        eye_sb = sb.tile([C4, C4], F32, tag="eye_sb")
        ones_sb = sb.tile([1, C4], F32, tag="ones_sb")

        # weights in (rounded to f32r via DVE copy)
        for w_d, w_sb in ((aps["w1_d"], w1_sb), (aps["w2_d"], w2_sb)):
            nc.sync.dma_start(wtmp_sb[:], w_d[:])
            nc.vector.tensor_copy(w_sb[:], wtmp_sb[:])
        nc.sync.dma_start(wtmp_sb[:, 0:128], aps["w3_d"][:])
        nc.vector.tensor_copy(w3_sb[:], wtmp_sb[:, 0:128])
        nc.sync.dma_start(wtmp_sb[0:C4, 0:C], aps["w4_d"][:])
        nc.vector.tensor_copy(w4_sb[:], wtmp_sb[0:C4, 0:C])
        nc.sync.dma_start(b123_sb[:], aps["b123_d"][:])
        nc.sync.dma_start(b4_sb[:], aps["b4_d"][:])
        masks.make_identity(nc, eye_sb[:])
        nc.vector.memset(ones_sb[:], 1.0)
        ones_col = sb.tile([128, N_JBLKS], F32, tag="ones_col")
        nc.vector.memset(ones_col[:], 1.0)
        nc.vector.tensor_copy(
            gt_sb[:].rearrange("p (j c) -> p j c", c=C4 + 1)
            [:, :, C4:C4 + 1],
            ones_col[:].rearrange("p (j c) -> p j c", c=1))

        # x in, chunked; round-copy to f32r
        NCH = 1024
        for c0 in range(0, 2 * N, NCH):
            nc.sync.dma_start(x_sb[:, c0:c0 + NCH], x_d[:, c0:c0 + NCH])
            nc.vector.tensor_copy(xr_sb[:, c0:c0 + NCH], x_sb[:, c0:c0 + NCH])

        # ---- phase A: theta / phi / g conv1x1; gT via PE transpose ----
        with tc.tile_pool(name="psA", bufs=2, space="PSUM") as psA:

            def conv(dst_sb, w_sb_, bias_col, m):
                for n in range(N_ITILES):
                    ps = psA.tile([128, NI], F32, tag="convps")
                    for k in range(2):
                        nc.tensor.matmul(
                            ps[0:m, :],
                            w_sb_[:, k * m:(k + 1) * m],
                            xr_sb[:, k * N + n * NI:k * N + (n + 1) * NI],
                            start=(k == 0), stop=(k == 1))
                    # bias-add + PSUM->SBUF (+ rounding) in one DVE op
                    nc.vector.tensor_scalar_add(
                        dst_sb[0:m, n * NI:(n + 1) * NI], ps[0:m, :],
                        b123_sb[0:m, bias_col:bias_col + 1])

            conv(ph_sb, w2_sb, 1, 128)   # phi first: S^T needs all of phi
            conv(th_sb, w1_sb, 0, 128)
            conv(g_sb, w3_sb, 2, C4)

            # gT: transpose g 128-col blocks -> [128, 64] each
            for j in range(N_JBLKS):
                tp = psA.tile([128, C4], F32, tag="tpps")
                nc.tensor.transpose(tp[:], g_sb[:, j * NJ:(j + 1) * NJ],
                                    eye_sb[:])
                nc.vector.tensor_copy(
                    gt_sb[:, j * (C4 + 1):j * (C4 + 1) + C4], tp[:])

        # ---- main loop ----
        SBANKS = cfg.get("stage_banks", 4)      # psum banks for S^T staging
        HALF = SBANKS // 2 * 512                # cols per staging half
        JPB = HALF // NI                        # j-blocks per exp batch
        ps_stage = st.enter_context(
            tc.tile_pool(name="ps_stage", bufs=2, space="PSUM"))
        ps_pv = st.enter_context(
            tc.tile_pool(name="ps_pv", bufs=2, space="PSUM"))
        ps_misc = st.enter_context(
            tc.tile_pool(name="ps_misc", bufs=1, space="PSUM"))
        pt_pool = st.enter_context(tc.tile_pool(name="pt", bufs=2))
        dv_pool = st.enter_context(tc.tile_pool(name="dv", bufs=2))
        y_pool = st.enter_context(tc.tile_pool(name="yp", bufs=3))

        batches = []
        j = 0
        while j < N_JBLKS:
            nb = min(JPB, N_JBLKS - j)
            batches.append(list(range(j, j + nb)))
            j += nb
        NB = len(batches)

        pvs = [None] * N_ITILES

        def emit_s(i, b):
            # row-packed pairs: even j-blocks on PE rows 0-63, odd on 64-127
            # (theta/phi are duplicated across halves) -> concurrent matmuls
            # and full-array activity for the HAM clock gate.
            stage_t = ps_stage.tile([128, HALF], F32, tag="stage",
                                    name=f"stage_{i}_{b}")
            half = stage_t[:, 0:len(batches[b]) * NI]
            for k, jb in enumerate(batches[b]):
                lo = (k % 2) * C4
                nc.tensor.matmul(
                    half[:, k * NI:(k + 1) * NI],
                    ph_sb[lo:lo + C4, jb * NJ:(jb + 1) * NJ],
                    th_sb[lo:lo + C4, i * NI:(i + 1) * NI],
                    start=True, stop=True)
            return half

        def emit_exp_pv(i, b, half):
            w = len(batches[b]) * NI
            pt = pt_pool.tile([128, HALF], MM, tag="pt")
            nc.scalar.activation(pt[:, 0:w], half[:], Exp)
            pv = pvs[i]
            for k, jb in enumerate(batches[b]):
                nc.tensor.matmul(
                    pv[0:C4 + 1, :],
                    gt_sb[:, jb * (C4 + 1):(jb + 1) * (C4 + 1)],
                    pt[:, k * NI:(k + 1) * NI],
                    start=(jb == 0), stop=(jb == N_JBLKS - 1))

        def emit_tail(i):
            pv = pvs[i]
            recip = dv_pool.tile([1, NI], F32, tag="recip")
            nc.vector.reciprocal(recip[:], pv[C4:C4 + 1, :])
            bc = ps_misc.tile([128, NI], F32, tag="misc")
            nc.tensor.matmul(bc[0:C4, :], ones_sb[:], recip[:],
                             start=True, stop=True)
            bcs = dv_pool.tile([C4, NI], F32, tag="bcs")
            nc.vector.tensor_copy(bcs[:], bc[0:C4, :])
            ag = dv_pool.tile([C4, NI], MM, tag="ag")
            nc.vector.tensor_tensor(ag[:], pv[0:C4, :], bcs[:], Alu.mult)
            for h in range(2):
                z = ps_misc.tile([128, NI], F32, tag="misc")
                nc.tensor.matmul(z[:], w4_sb[:, h * 128:(h + 1) * 128],
                                 ag[:], start=True, stop=True)
                yt = y_pool.tile([128, NI], F32, tag="yt")
                # y = (z + b4) + x
                nc.vector.scalar_tensor_tensor(
                    yt[:], z[:], b4_sb[:, h:h + 1],
                    x_sb[:, h * N + i * NI:h * N + (i + 1) * NI],
                    Alu.add, Alu.add)
                nc.sync.dma_start(
                    y_d[h * 128:(h + 1) * 128, i * NI:(i + 1) * NI], yt[:])

        # software-pipelined emission: S(b+1) lands before exp/PV(b) on the
        # PE stream; the previous i-tile's tail is deferred past the first S
        # batch of the next i-tile so ACT never waits on the tail chain.
        pending_tail = None
        TAIL_AT = 7   # defer prev tail this many batches into the next i-tile
        for i in range(N_ITILES):
            pvs[i] = ps_pv.tile([128, NI], F32, tag="pv", name=f"pv{i}")
            halves = [None] * NB
            halves[0] = emit_s(i, 0)
            for b in range(NB):
                if b + 1 < NB:
                    halves[b + 1] = emit_s(i, b + 1)
                emit_exp_pv(i, b, halves[b])
                if b == TAIL_AT and pending_tail is not None:
                    emit_tail(pending_tail)
                    pending_tail = None
            pending_tail = i
        emit_tail(pending_tail)


def _prepare_core_inputs(x_b, W1, b1, W2, b2, W3, b3, W4, b4):
    def ktile(wT, m):
        # [256, m] -> [128, 2*m] (two k-tiles side by side)
        return np.ascontiguousarray(
            wT.reshape(2, 128, m).transpose(1, 0, 2).reshape(128, 2 * m))

    def dup(wT):
        # duplicate output channels across both halves: [256,64] -> [256,128]
        return np.concatenate([wT, wT], axis=1)

    z64 = np.zeros(C4, np.float32)
    return {
        "x": np.ascontiguousarray(
            x_b.reshape(2, 128, N).transpose(1, 0, 2).reshape(128, 2 * N)),
        "w1t": ktile(dup(W1.T), 128), "w2t": ktile(dup(W2.T), 128),
        "w3t": ktile(W3.T, C4),
        "w4t": np.ascontiguousarray(W4.T),
        "b123": np.ascontiguousarray(
            np.stack([np.r_[b1, b1], np.r_[b2, b2], np.r_[b3, z64]], axis=1)),
        "b4c": np.ascontiguousarray(b4.reshape(2, 128).T),
    }


def kernel(x, W1, b1, W2, b2, W3, b3, W4, b4, _trace=False, _cfg=None):
    from concourse import bass_utils

    cfg = dict(_cfg or {})
    key = tuple(sorted(cfg.items()))
    if key not in _CACHE:
        _CACHE[key] = _build(cfg)
    nc = _CACHE[key]

    x = np.asarray(x, dtype=np.float32)
    xf = x.reshape(B, C, N)
    args = [np.asarray(a, dtype=np.float32)
            for a in (W1, b1, W2, b2, W3, b3, W4, b4)]
    in_maps = [_prepare_core_inputs(xf[b], *args) for b in range(B)]
    res = bass_utils.run_bass_kernel_spmd(
        nc, in_maps, core_ids=list(range(B)), trace=_trace)
    out = np.stack([res.results[b]["y"].reshape(C, H, W) for b in range(B)])
    if _trace:
        kernel.last_exec_time_ns = res.exec_time_ns
    return out


# revision 21
# speedup vs baseline: 1.2818x; 1.2326x over previous
"""Trainium2 Bass kernel for nn_Atten_Block (non-local attention block).

Reference computation per batch element b (C=256, C4=64, H=W=64, N=4096):
    theta = W1 @ x + b1          [C4, N]
    phi   = W2 @ x + b2          [C4, N]
    g     = W3 @ x + b3          [C4, N]
    S     = theta^T @ phi        [N, N]
    A     = softmax(S, axis=-1)
    attn_g[c,i] = sum_j g[c,j] A[i,j]
    y     = x + W4 @ attn_g + b4

Sharding: data-parallel over batch B=8 across the 8 NeuronCores (one batch
element per core).

Per-core algorithm (engine-balanced around the ScalarE exp bottleneck):
  - S is computed TRANSPOSED: S^T tile [j=128, i=512] = phi_jblk.T @ theta_i
    so that softmax normalization and the PV matmul need no transposes:
      P^T = exp(S^T)  (no max-subtraction: |S| <= ~65 < 88, safe in fp32)
      pv[c,i] = sum_j gT[j,c] P^T[j,i]  via matmul with lhsT = [gT | ones]
    The appended ones column makes pv row 64 the softmax denominators l[i].
  - attn_g = pv[0:64] * (1/l) broadcast via a K=1 ones matmul.
  - y = x + W4 @ attn_g + b4 fused in one DVE op per tile.

Matmuls run in float32r (1 cyc/row vs fp32's 4) — producers round to f32r.
"""

import sys
from contextlib import ExitStack

import numpy as np

if "/opt/trn_rl_repo" not in sys.path:
    sys.path.insert(0, "/opt/trn_rl_repo")

C = 256
C4 = 64
B = 8
H = W = 64
N = H * W          # 4096
NI = 512           # i-tile width (matmul free dim)
NJ = 128           # j-block (S^T partition dim)
N_ITILES = N // NI   # 8
N_JBLKS = N // NJ    # 32

_CACHE = {}


def _build(cfg):
    import concourse.tile as tile
    from concourse import bacc, mybir

    F32 = mybir.dt.float32

    nc = bacc.Bacc("TRN2", target_bir_lowering=False, debug=False,
                   num_devices=B)

    aps = dict(
        x_d=nc.dram_tensor("x", [128, 2 * N], F32, kind="ExternalInput").ap(),
        w1_d=nc.dram_tensor("w1t", [128, 256], F32, kind="ExternalInput").ap(),
        w2_d=nc.dram_tensor("w2t", [128, 256], F32, kind="ExternalInput").ap(),
        w3_d=nc.dram_tensor("w3t", [128, 128], F32, kind="ExternalInput").ap(),
        w4_d=nc.dram_tensor("w4t", [C4, C], F32, kind="ExternalInput").ap(),
        b123_d=nc.dram_tensor("b123", [128, 3], F32, kind="ExternalInput").ap(),
        b4_d=nc.dram_tensor("b4c", [128, 2], F32, kind="ExternalInput").ap(),
        y_d=nc.dram_tensor("y", [C, N], F32, kind="ExternalOutput").ap(),
    )

    with tile.TileContext(nc) as tc:
        _body(nc, tc, cfg, aps)
    nc.compile()
    return nc


def _body(nc, tc, cfg, aps):
    import concourse.tile as tile  # noqa: F401
    from concourse import masks, mybir
    from concourse.alu_op_type import AluOpType as Alu

    F32 = mybir.dt.float32
    F32R = mybir.dt.float32r
    MM = F32R if cfg.get("f32r", True) else F32
    Exp = mybir.ActivationFunctionType.Exp

    x_d, y_d = aps["x_d"], aps["y_d"]

    with ExitStack() as st:
        sb = st.enter_context(tc.tile_pool(name="sb", bufs=1))

        # ---- static SBUF tensors ----
        x_sb = sb.tile([128, 2 * N], F32, tag="x_sb")      # residual source
        xr_sb = sb.tile([128, 2 * N], MM, tag="xr_sb")     # rounded for matmul
        # theta/phi duplicated across both partition halves (rows 64-127 =
        # rows 0-63) so S^T matmul pairs can row-pack the full PE array.
        th_sb = sb.tile([128, N], MM, tag="th_sb")         # theta (dup)
        ph_sb = sb.tile([128, N], MM, tag="ph_sb")         # phi (dup)
        g_sb = sb.tile([C4, N], F32, tag="g_sb")           # g (pre-transpose)
        gt_sb = sb.tile([128, N_JBLKS * (C4 + 1)], MM, tag="gt_sb")  # [gT|1]
        w1_sb = sb.tile([128, 256], MM, tag="w1_sb")       # dup-M k-tiles
        w2_sb = sb.tile([128, 256], MM, tag="w2_sb")
        w3_sb = sb.tile([128, 128], MM, tag="w3_sb")
        w4_sb = sb.tile([C4, C], MM, tag="w4_sb")
        wtmp_sb = sb.tile([128, C], F32, tag="wtmp_sb")
        b123_sb = sb.tile([128, 3], F32, tag="b123_sb")
        b4_sb = sb.tile([128, 2], F32, tag="b4_sb")
        eye_sb = sb.tile([C4, C4], F32, tag="eye_sb")
        ones_sb = sb.tile([1, C4], F32, tag="ones_sb")

        # weights in (rounded to f32r via DVE copy)
        for w_d, w_sb in ((aps["w1_d"], w1_sb), (aps["w2_d"], w2_sb)):
            nc.sync.dma_start(wtmp_sb[:], w_d[:])
            nc.vector.tensor_copy(w_sb[:], wtmp_sb[:])
        nc.sync.dma_start(wtmp_sb[:, 0:128], aps["w3_d"][:])
        nc.vector.tensor_copy(w3_sb[:], wtmp_sb[:, 0:128])
        nc.sync.dma_start(wtmp_sb[0:C4, 0:C], aps["w4_d"][:])
        nc.vector.tensor_copy(w4_sb[:], wtmp_sb[0:C4, 0:C])
        nc.sync.dma_start(b123_sb[:], aps["b123_d"][:])
        nc.sync.dma_start(b4_sb[:], aps["b4_d"][:])
        masks.make_identity(nc, eye_sb[:])
        nc.vector.memset(ones_sb[:], 1.0)
        ones_col = sb.tile([128, N_JBLKS], F32, tag="ones_col")
        nc.vector.memset(ones_col[:], 1.0)
        nc.vector.tensor_copy(
            gt_sb[:].rearrange("p (j c) -> p j c", c=C4 + 1)
            [:, :, C4:C4 + 1],
            ones_col[:].rearrange("p (j c) -> p j c", c=1))

        # x in, chunked; round-copy to f32r
        NCH = 1024
        for c0 in range(0, 2 * N, NCH):
            nc.sync.dma_start(x_sb[:, c0:c0 + NCH], x_d[:, c0:c0 + NCH])
            nc.vector.tensor_copy(xr_sb[:, c0:c0 + NCH], x_sb[:, c0:c0 + NCH])

        # ---- phase A: theta / phi / g conv1x1; gT via PE transpose ----
        with tc.tile_pool(name="psA", bufs=2, space="PSUM") as psA:

            def conv(dst_sb, w_sb_, bias_col, m):
                for n in range(N_ITILES):
                    ps = psA.tile([128, NI], F32, tag="convps")
                    for k in range(2):
                        nc.tensor.matmul(
                            ps[0:m, :],
                            w_sb_[:, k * m:(k + 1) * m],
                            xr_sb[:, k * N + n * NI:k * N + (n + 1) * NI],
                            start=(k == 0), stop=(k == 1))
                    # bias-add + PSUM->SBUF (+ rounding) in one DVE op
                    nc.vector.tensor_scalar_add(
                        dst_sb[0:m, n * NI:(n + 1) * NI], ps[0:m, :],
                        b123_sb[0:m, bias_col:bias_col + 1])

            conv(ph_sb, w2_sb, 1, 128)   # phi first: S^T needs all of phi
            conv(th_sb, w1_sb, 0, 128)
            conv(g_sb, w3_sb, 2, C4)

            # gT: transpose g 128-col blocks -> [128, 64] each
            for j in range(N_JBLKS):
                tp = psA.tile([128, C4], F32, tag="tpps")
                nc.tensor.transpose(tp[:], g_sb[:, j * NJ:(j + 1) * NJ],
                                    eye_sb[:])
                nc.vector.tensor_copy(
                    gt_sb[:, j * (C4 + 1):j * (C4 + 1) + C4], tp[:])

        # ---- main loop ----
        SBANKS = cfg.get("stage_banks", 4)      # psum banks for S^T staging
        HALF = SBANKS // 2 * 512                # cols per staging half
        JPB = HALF // NI                        # j-blocks per exp batch
        ps_stage = st.enter_context(
            tc.tile_pool(name="ps_stage", bufs=2, space="PSUM"))
        ps_pv = st.enter_context(
            tc.tile_pool(name="ps_pv", bufs=2, space="PSUM"))
        ps_misc = st.enter_context(
            tc.tile_pool(name="ps_misc", bufs=1, space="PSUM"))
        pt_pool = st.enter_context(tc.tile_pool(name="pt", bufs=2))
        dv_pool = st.enter_context(tc.tile_pool(name="dv", bufs=2))
        y_pool = st.enter_context(tc.tile_pool(name="yp", bufs=3))

        batches = []
        j = 0
        while j < N_JBLKS:
            nb = min(JPB, N_JBLKS - j)
            batches.append(list(range(j, j + nb)))
            j += nb
        NB = len(batches)

        pvs = [None] * N_ITILES

        def emit_s(i, b):
            # row-packed pairs: even j-blocks on PE rows 0-63, odd on 64-127
            # (theta/phi are duplicated across halves) -> concurrent matmuls
            # and full-array activity for the HAM clock gate.
            stage_t = ps_stage.tile([128, HALF], F32, tag="stage",
                                    name=f"stage_{i}_{b}")
            half = stage_t[:, 0:len(batches[b]) * NI]
            for k, jb in enumerate(batches[b]):
                lo = (k % 2) * C4
                nc.tensor.matmul(
                    half[:, k * NI:(k + 1) * NI],
                    ph_sb[lo:lo + C4, jb * NJ:(jb + 1) * NJ],
                    th_sb[lo:lo + C4, i * NI:(i + 1) * NI],
                    start=True, stop=True)
            return half

        def emit_exp_pv(i, b, half):
            w = len(batches[b]) * NI
            pt = pt_pool.tile([128, HALF], MM, tag="pt")
            nc.scalar.activation(pt[:, 0:w], half[:], Exp)
            pv = pvs[i]
            for k, jb in enumerate(batches[b]):
                nc.tensor.matmul(
                    pv[0:C4 + 1, :],
                    gt_sb[:, jb * (C4 + 1):(jb + 1) * (C4 + 1)],
                    pt[:, k * NI:(k + 1) * NI],
                    start=(jb == 0), stop=(jb == N_JBLKS - 1))

        def emit_tail(i):
            pv = pvs[i]
            lrow = dv_pool.tile([1, NI], F32, tag="lrow")
            nc.vector.tensor_copy(lrow[:], pv[C4:C4 + 1, :])
            recip = dv_pool.tile([1, NI], F32, tag="recip")
            nc.vector.reciprocal_approx_fast(recip[:], lrow[:])
            bc = ps_misc.tile([128, NI], F32, tag="misc")
            nc.tensor.matmul(bc[0:C4, :], ones_sb[:], recip[:],
                             start=True, stop=True)
            bcs = dv_pool.tile([C4, NI], F32, tag="bcs")
            nc.vector.tensor_copy(bcs[:], bc[0:C4, :])
            ag = dv_pool.tile([C4, NI], MM, tag="ag")
            nc.vector.tensor_tensor(ag[:], pv[0:C4, :], bcs[:], Alu.mult)
            for h in range(2):
                z = ps_misc.tile([128, NI], F32, tag="misc")
                nc.tensor.matmul(z[:], w4_sb[:, h * 128:(h + 1) * 128],
                                 ag[:], start=True, stop=True)
                yt = y_pool.tile([128, NI], F32, tag="yt")
                # y = (z + b4) + x
                nc.vector.scalar_tensor_tensor(
                    yt[:], z[:], b4_sb[:, h:h + 1],
                    x_sb[:, h * N + i * NI:h * N + (i + 1) * NI],
                    Alu.add, Alu.add)
                nc.sync.dma_start(
                    y_d[h * 128:(h + 1) * 128, i * NI:(i + 1) * NI], yt[:])

        # software-pipelined emission: S(b+1) lands before exp/PV(b) on the
        # PE stream; the previous i-tile's tail is deferred past the first S
        # batch of the next i-tile so ACT never waits on the tail chain.
        pending_tail = None
        TAIL_AT = 7   # defer prev tail this many batches into the next i-tile
        for i in range(N_ITILES):
            pvs[i] = ps_pv.tile([128, NI], F32, tag="pv", name=f"pv{i}")
            halves = [None] * NB
            halves[0] = emit_s(i, 0)
            for b in range(NB):
                if b + 1 < NB:
                    halves[b + 1] = emit_s(i, b + 1)
                emit_exp_pv(i, b, halves[b])
                if b == TAIL_AT and pending_tail is not None:
                    emit_tail(pending_tail)
                    pending_tail = None
            pending_tail = i
        emit_tail(pending_tail)


def _prepare_core_inputs(x_b, W1, b1, W2, b2, W3, b3, W4, b4):
    def ktile(wT, m):
        # [256, m] -> [128, 2*m] (two k-tiles side by side)
        return np.ascontiguousarray(
            wT.reshape(2, 128, m).transpose(1, 0, 2).reshape(128, 2 * m))

    def dup(wT):
        # duplicate output channels across both halves: [256,64] -> [256,128]
        return np.concatenate([wT, wT], axis=1)

    z64 = np.zeros(C4, np.float32)
    return {
        "x": np.ascontiguousarray(
            x_b.reshape(2, 128, N).transpose(1, 0, 2).reshape(128, 2 * N)),
        "w1t": ktile(dup(W1.T), 128), "w2t": ktile(dup(W2.T), 128),
        "w3t": ktile(W3.T, C4),
        "w4t": np.ascontiguousarray(W4.T),
        "b123": np.ascontiguousarray(
            np.stack([np.r_[b1, b1], np.r_[b2, b2], np.r_[b3, z64]], axis=1)),
        "b4c": np.ascontiguousarray(b4.reshape(2, 128).T),
    }


def kernel(x, W1, b1, W2, b2, W3, b3, W4, b4, _trace=False, _cfg=None):
    from concourse import bass_utils

    cfg = dict(_cfg or {})
    key = tuple(sorted(cfg.items()))
    if key not in _CACHE:
        _CACHE[key] = _build(cfg)
    nc = _CACHE[key]

    x = np.asarray(x, dtype=np.float32)
    xf = x.reshape(B, C, N)
    args = [np.asarray(a, dtype=np.float32)
            for a in (W1, b1, W2, b2, W3, b3, W4, b4)]
    in_maps = [_prepare_core_inputs(xf[b], *args) for b in range(B)]
    res = bass_utils.run_bass_kernel_spmd(
        nc, in_maps, core_ids=list(range(B)), trace=_trace)
    out = np.stack([res.results[b]["y"].reshape(C, H, W) for b in range(B)])
    if _trace:
        kernel.last_exec_time_ns = res.exec_time_ns
    return out


# revision 25
# speedup vs baseline: 1.3033x; 1.0168x over previous
"""Trainium2 Bass kernel for nn_Atten_Block (non-local attention block).

Reference computation per batch element b (C=256, C4=64, H=W=64, N=4096):
    theta = W1 @ x + b1          [C4, N]
    phi   = W2 @ x + b2          [C4, N]
    g     = W3 @ x + b3          [C4, N]
    S     = theta^T @ phi        [N, N]
    A     = softmax(S, axis=-1)
    attn_g[c,i] = sum_j g[c,j] A[i,j]
    y     = x + W4 @ attn_g + b4

Sharding: data-parallel over batch B=8 across the 8 NeuronCores (one batch
element per core).

Per-core algorithm (engine-balanced around the ScalarE exp bottleneck):
  - S is computed TRANSPOSED: S^T tile [j=128, i=512] = phi_jblk.T @ theta_i
    so that softmax normalization and the PV matmul need no transposes:
      P^T = exp(S^T)  (no max-subtraction: |S| <= ~65 < 88, safe in fp32)
      pv[c,i] = sum_j gT[j,c] P^T[j,i]  via matmul with lhsT = [gT | ones]
    The appended ones column makes pv row 64 the softmax denominators l[i].
  - attn_g = pv[0:64] * (1/l) broadcast via a K=1 ones matmul.
  - y = x + W4 @ attn_g + b4 fused in one DVE op per tile.

Matmuls run in float32r (1 cyc/row vs fp32's 4) — producers round to f32r.
"""

import sys
from contextlib import ExitStack

import numpy as np

if "/opt/trn_rl_repo" not in sys.path:
    sys.path.insert(0, "/opt/trn_rl_repo")

C = 256
C4 = 64
B = 8
H = W = 64
N = H * W          # 4096
NI = 512           # i-tile width (matmul free dim)
NJ = 128           # j-block (S^T partition dim)
N_ITILES = N // NI   # 8
N_JBLKS = N // NJ    # 32

_CACHE = {}


def _build(cfg):
    import concourse.tile as tile
    from concourse import bacc, mybir

    F32 = mybir.dt.float32

    nc = bacc.Bacc("TRN2", target_bir_lowering=False, debug=False,
                   num_devices=B)

    F32R = mybir.dt.float32r
    MMD = F32R if cfg.get("f32r", True) else F32
    aps = dict(
        x_d=nc.dram_tensor("x", [128, 2 * N], MMD, kind="ExternalInput").ap(),
        w1_d=nc.dram_tensor("w1t", [128, 256], MMD, kind="ExternalInput").ap(),
        w2_d=nc.dram_tensor("w2t", [128, 256], MMD, kind="ExternalInput").ap(),
        w3_d=nc.dram_tensor("w3t", [128, 128], MMD, kind="ExternalInput").ap(),
        w4_d=nc.dram_tensor("w4t", [C4, C], MMD, kind="ExternalInput").ap(),
        b123_d=nc.dram_tensor("b123", [128, 3], F32, kind="ExternalInput").ap(),
        b4_d=nc.dram_tensor("b4c", [128, 2], F32, kind="ExternalInput").ap(),
        y_d=nc.dram_tensor("y", [C, N], F32, kind="ExternalOutput").ap(),
    )

    with tile.TileContext(nc) as tc:
        _body(nc, tc, cfg, aps)
    nc.compile()
    return nc


def _body(nc, tc, cfg, aps):
    import concourse.tile as tile  # noqa: F401
    from concourse import masks, mybir
    from concourse.alu_op_type import AluOpType as Alu

    F32 = mybir.dt.float32
    F32R = mybir.dt.float32r
    MM = F32R if cfg.get("f32r", True) else F32
    Exp = mybir.ActivationFunctionType.Exp

    x_d, y_d = aps["x_d"], aps["y_d"]

    with ExitStack() as st:
        sb = st.enter_context(tc.tile_pool(name="sb", bufs=1))

        # ---- static SBUF tensors ----
        x_sb = sb.tile([128, 2 * N], F32, tag="x_sb")      # residual source
        xr_sb = sb.tile([128, 2 * N], MM, tag="xr_sb")     # rounded for matmul
        # theta/phi duplicated across both partition halves (rows 64-127 =
        # rows 0-63) so S^T matmul pairs can row-pack the full PE array.
        th_sb = sb.tile([128, N], MM, tag="th_sb")         # theta (dup)
        ph_sb = sb.tile([128, N], MM, tag="ph_sb")         # phi (dup)
        g_sb = sb.tile([C4, N], F32, tag="g_sb")           # g (pre-transpose)
        gt_sb = sb.tile([128, N_JBLKS * (C4 + 1)], MM, tag="gt_sb")  # [gT|1]
        w1_sb = sb.tile([128, 256], MM, tag="w1_sb")       # dup-M k-tiles
        w2_sb = sb.tile([128, 256], MM, tag="w2_sb")
        w3_sb = sb.tile([128, 128], MM, tag="w3_sb")
        w4_sb = sb.tile([C4, C], MM, tag="w4_sb")
        b123_sb = sb.tile([128, 3], F32, tag="b123_sb")
        b4_sb = sb.tile([128, 2], F32, tag="b4_sb")
        eye_sb = sb.tile([C4, C4], F32, tag="eye_sb")
        ones_sb = sb.tile([1, C4], F32, tag="ones_sb")

        # weights in — direct DMA to f32r tiles (dtype-matched, no cast)
        nc.sync.dma_start(w1_sb[:], aps["w1_d"][:])
        nc.sync.dma_start(w2_sb[:], aps["w2_d"][:])
        nc.sync.dma_start(w3_sb[:], aps["w3_d"][:])
        nc.sync.dma_start(w4_sb[:], aps["w4_d"][:])
        nc.sync.dma_start(b123_sb[:], aps["b123_d"][:])
        nc.sync.dma_start(b4_sb[:], aps["b4_d"][:])
        masks.make_identity(nc, eye_sb[:])
        nc.vector.memset(ones_sb[:], 1.0)
        ones_col = sb.tile([128, N_JBLKS], F32, tag="ones_col")
        nc.vector.memset(ones_col[:], 1.0)
        nc.vector.tensor_copy(
            gt_sb[:].rearrange("p (j c) -> p j c", c=C4 + 1)
            [:, :, C4:C4 + 1],
            ones_col[:].rearrange("p (j c) -> p j c", c=1))

        # x in: f32r copy for matmuls (per n-tile, both k halves so the conv
        # pipeline fills as chunks land) + fp32 copy for the residual adds
        # (only consumed at tails, trickles in the background).
        for n in range(N_ITILES):
            for k in range(2):
                c0 = k * N + n * NI
                nc.sync.dma_start(xr_sb[:, c0:c0 + NI], x_d[:, c0:c0 + NI])
        for c0 in range(0, 2 * N, 1024):
            nc.sync.dma_start(x_sb[:, c0:c0 + 1024],
                              x_d[:, c0:c0 + 1024].bitcast(F32))

        # ---- phase A: theta / phi / g conv1x1; gT via PE transpose ----
        with tc.tile_pool(name="psA", bufs=2, space="PSUM") as psA:

            def conv(dst_sb, w_sb_, bias_col, m, n):
                ps = psA.tile([128, NI], F32, tag="convps")
                for k in range(2):
                    nc.tensor.matmul(
                        ps[0:m, :],
                        w_sb_[:, k * m:(k + 1) * m],
                        xr_sb[:, k * N + n * NI:k * N + (n + 1) * NI],
                        start=(k == 0), stop=(k == 1))
                # bias-add + PSUM->SBUF (+ rounding) in one DVE op
                nc.vector.tensor_scalar_add(
                    dst_sb[0:m, n * NI:(n + 1) * NI], ps[0:m, :],
                    b123_sb[0:m, bias_col:bias_col + 1])

            for n in range(N_ITILES):
                conv(ph_sb, w2_sb, 1, 128, n)
                conv(th_sb, w1_sb, 0, 128, n)
                conv(g_sb, w3_sb, 2, C4, n)
                # gT: transpose fresh g 128-col blocks -> [128, 64] each
                for j in range(4 * n, 4 * n + 4):
                    tp = psA.tile([128, C4], F32, tag="tpps")
                    nc.tensor.transpose(tp[:], g_sb[:, j * NJ:(j + 1) * NJ],
                                        eye_sb[:])
                    nc.vector.tensor_copy(
                        gt_sb[:, j * (C4 + 1):j * (C4 + 1) + C4], tp[:])

        # ---- main loop ----
        SBANKS = cfg.get("stage_banks", 4)      # psum banks for S^T staging
        HALF = SBANKS // 2 * 512                # cols per staging half
        JPB = HALF // NI                        # j-blocks per exp batch
        ps_stage = st.enter_context(
            tc.tile_pool(name="ps_stage", bufs=2, space="PSUM"))
        ps_pv = st.enter_context(
            tc.tile_pool(name="ps_pv", bufs=2, space="PSUM"))
        ps_misc = st.enter_context(
            tc.tile_pool(name="ps_misc", bufs=1, space="PSUM"))
        pt_pool = st.enter_context(tc.tile_pool(name="pt", bufs=2))
        dv_pool = st.enter_context(tc.tile_pool(name="dv", bufs=2))
        y_pool = st.enter_context(tc.tile_pool(name="yp", bufs=3))

        batches = []
        j = 0
        while j < N_JBLKS:
            nb = min(JPB, N_JBLKS - j)
            batches.append(list(range(j, j + nb)))
            j += nb
        NB = len(batches)

        pvs = [None] * N_ITILES

        def emit_s(i, b):
            # row-packed pairs: even j-blocks on PE rows 0-63, odd on 64-127
            # (theta/phi are duplicated across halves) -> concurrent matmuls
            # and full-array activity for the HAM clock gate.
            stage_t = ps_stage.tile([128, HALF], F32, tag="stage",
                                    name=f"stage_{i}_{b}")
            half = stage_t[:, 0:len(batches[b]) * NI]
            for k, jb in enumerate(batches[b]):
                lo = (k % 2) * C4
                nc.tensor.matmul(
                    half[:, k * NI:(k + 1) * NI],
                    ph_sb[lo:lo + C4, jb * NJ:(jb + 1) * NJ],
                    th_sb[lo:lo + C4, i * NI:(i + 1) * NI],
                    start=True, stop=True)
            return half

        def emit_exp_pv(i, b, half):
            w = len(batches[b]) * NI
            pt = pt_pool.tile([128, HALF], MM, tag="pt")
            nc.scalar.activation(pt[:, 0:w], half[:], Exp)
            pv = pvs[i]
            for k, jb in enumerate(batches[b]):
                nc.tensor.matmul(
                    pv[0:C4 + 1, :],
                    gt_sb[:, jb * (C4 + 1):(jb + 1) * (C4 + 1)],
                    pt[:, k * NI:(k + 1) * NI],
                    start=(jb == 0), stop=(jb == N_JBLKS - 1))

        def emit_tail(i):
            pv = pvs[i]
            lrow = dv_pool.tile([1, NI], F32, tag="lrow")
            nc.vector.tensor_copy(lrow[:], pv[C4:C4 + 1, :])
            recip = dv_pool.tile([1, NI], F32, tag="recip")
            nc.vector.reciprocal_approx_fast(recip[:], lrow[:])
            bc = ps_misc.tile([128, NI], F32, tag="misc")
            nc.tensor.matmul(bc[0:C4, :], ones_sb[:], recip[:],
                             start=True, stop=True)
            bcs = dv_pool.tile([C4, NI], F32, tag="bcs")
            nc.vector.tensor_copy(bcs[:], bc[0:C4, :])
            ag = dv_pool.tile([C4, NI], MM, tag="ag")
            nc.vector.tensor_tensor(ag[:], pv[0:C4, :], bcs[:], Alu.mult)
            for h in range(2):
                z = ps_misc.tile([128, NI], F32, tag="misc")
                nc.tensor.matmul(z[:], w4_sb[:, h * 128:(h + 1) * 128],
                                 ag[:], start=True, stop=True)
                yt = y_pool.tile([128, NI], F32, tag="yt")
                # y = (z + b4) + x
                nc.vector.scalar_tensor_tensor(
                    yt[:], z[:], b4_sb[:, h:h + 1],
                    x_sb[:, h * N + i * NI:h * N + (i + 1) * NI],
                    Alu.add, Alu.add)
                nc.sync.dma_start(
                    y_d[h * 128:(h + 1) * 128, i * NI:(i + 1) * NI], yt[:])

        # software-pipelined emission: S(b+1) lands before exp/PV(b) on the
        # PE stream; the previous i-tile's tail is deferred past the first S
        # batch of the next i-tile so ACT never waits on the tail chain.
        pending_tail = None
        TAIL_AT = 7   # defer prev tail this many batches into the next i-tile
        for i in range(N_ITILES):
            pvs[i] = ps_pv.tile([128, NI], F32, tag="pv", name=f"pv{i}")
            halves = [None] * NB
            halves[0] = emit_s(i, 0)
            for b in range(NB):
                if b + 1 < NB:
                    halves[b + 1] = emit_s(i, b + 1)
                emit_exp_pv(i, b, halves[b])
                if b == TAIL_AT and pending_tail is not None:
                    emit_tail(pending_tail)
                    pending_tail = None
            pending_tail = i
        emit_tail(pending_tail)


def _prepare_core_inputs(x_b, W1, b1, W2, b2, W3, b3, W4, b4):
    def ktile(wT, m):
        # [256, m] -> [128, 2*m] (two k-tiles side by side)
        return np.ascontiguousarray(
            wT.reshape(2, 128, m).transpose(1, 0, 2).reshape(128, 2 * m))

    def dup(wT):
        # duplicate output channels across both halves: [256,64] -> [256,128]
        return np.concatenate([wT, wT], axis=1)

    z64 = np.zeros(C4, np.float32)
    return {
        "x": np.ascontiguousarray(
            x_b.reshape(2, 128, N).transpose(1, 0, 2).reshape(128, 2 * N)),
        "w1t": ktile(dup(W1.T), 128), "w2t": ktile(dup(W2.T), 128),
        "w3t": ktile(W3.T, C4),
        "w4t": np.ascontiguousarray(W4.T),
        "b123": np.ascontiguousarray(
            np.stack([np.r_[b1, b1], np.r_[b2, b2], np.r_[b3, z64]], axis=1)),
        "b4c": np.ascontiguousarray(b4.reshape(2, 128).T),
    }


def kernel(x, W1, b1, W2, b2, W3, b3, W4, b4, _trace=False, _cfg=None):
    from concourse import bass_utils

    cfg = dict(_cfg or {})
    key = tuple(sorted(cfg.items()))
    if key not in _CACHE:
        _CACHE[key] = _build(cfg)
    nc = _CACHE[key]

    x = np.asarray(x, dtype=np.float32)
    xf = x.reshape(B, C, N)
    args = [np.asarray(a, dtype=np.float32)
            for a in (W1, b1, W2, b2, W3, b3, W4, b4)]
    in_maps = [_prepare_core_inputs(xf[b], *args) for b in range(B)]
    res = bass_utils.run_bass_kernel_spmd(
        nc, in_maps, core_ids=list(range(B)), trace=_trace)
    out = np.stack([res.results[b]["y"].reshape(C, H, W) for b in range(B)])
    if _trace:
        kernel.last_exec_time_ns = res.exec_time_ns
    return out


# revision 26
# speedup vs baseline: 1.3037x; 1.0003x over previous
"""Trainium2 Bass kernel for nn_Atten_Block (non-local attention block).

Reference computation per batch element b (C=256, C4=64, H=W=64, N=4096):
    theta = W1 @ x + b1          [C4, N]
    phi   = W2 @ x + b2          [C4, N]
    g     = W3 @ x + b3          [C4, N]
    S     = theta^T @ phi        [N, N]
    A     = softmax(S, axis=-1)
    attn_g[c,i] = sum_j g[c,j] A[i,j]
    y     = x + W4 @ attn_g + b4

Sharding: data-parallel over batch B=8 across the 8 NeuronCores (one batch
element per core).

Per-core algorithm (engine-balanced around the ScalarE exp bottleneck):
  - S is computed TRANSPOSED: S^T tile [j=128, i=512] = phi_jblk.T @ theta_i
    so that softmax normalization and the PV matmul need no transposes:
      P^T = exp(S^T)  (no max-subtraction: |S| <= ~65 < 88, safe in fp32)
      pv[c,i] = sum_j gT[j,c] P^T[j,i]  via matmul with lhsT = [gT | ones]
    The appended ones column makes pv row 64 the softmax denominators l[i].
  - attn_g = pv[0:64] * (1/l) broadcast via a K=1 ones matmul.
  - y = x + W4 @ attn_g + b4 fused in one DVE op per tile.

Matmuls run in float32r (1 cyc/row vs fp32's 4) — producers round to f32r.
"""

import sys
from contextlib import ExitStack

import numpy as np

if "/opt/trn_rl_repo" not in sys.path:
    sys.path.insert(0, "/opt/trn_rl_repo")

C = 256
C4 = 64
B = 8
H = W = 64
N = H * W          # 4096
NI = 512           # i-tile width (matmul free dim)
NJ = 128           # j-block (S^T partition dim)
N_ITILES = N // NI   # 8
N_JBLKS = N // NJ    # 32

_CACHE = {}


def _build(cfg):
    import concourse.tile as tile
    from concourse import bacc, mybir

    F32 = mybir.dt.float32

    nc = bacc.Bacc("TRN2", target_bir_lowering=False, debug=False,
                   num_devices=B)

    F32R = mybir.dt.float32r
    MMD = F32R if cfg.get("f32r", True) else F32
    aps = dict(
        x_d=nc.dram_tensor("x", [128, 2 * N], MMD, kind="ExternalInput").ap(),
        w1_d=nc.dram_tensor("w1t", [128, 256], MMD, kind="ExternalInput").ap(),
        w2_d=nc.dram_tensor("w2t", [128, 256], MMD, kind="ExternalInput").ap(),
        w3_d=nc.dram_tensor("w3t", [128, 128], MMD, kind="ExternalInput").ap(),
        w4_d=nc.dram_tensor("w4t", [C4, C], MMD, kind="ExternalInput").ap(),
        b123_d=nc.dram_tensor("b123", [128, 3], F32, kind="ExternalInput").ap(),
        b4_d=nc.dram_tensor("b4c", [128, 2], F32, kind="ExternalInput").ap(),
        y_d=nc.dram_tensor("y", [C, N], F32, kind="ExternalOutput").ap(),
    )

    with tile.TileContext(nc) as tc:
        _body(nc, tc, cfg, aps)
    nc.compile()
    return nc


def _body(nc, tc, cfg, aps):
    import concourse.tile as tile  # noqa: F401
    from concourse import masks, mybir
    from concourse.alu_op_type import AluOpType as Alu

    F32 = mybir.dt.float32
    F32R = mybir.dt.float32r
    MM = F32R if cfg.get("f32r", True) else F32
    Exp = mybir.ActivationFunctionType.Exp

    x_d, y_d = aps["x_d"], aps["y_d"]

    with ExitStack() as st:
        sb = st.enter_context(tc.tile_pool(name="sb", bufs=1))

        # ---- static SBUF tensors ----
        x_sb = sb.tile([128, 2 * N], F32, tag="x_sb")      # residual source
        xr_sb = sb.tile([128, 2 * N], MM, tag="xr_sb")     # rounded for matmul
        # theta/phi duplicated across both partition halves (rows 64-127 =
        # rows 0-63) so S^T matmul pairs can row-pack the full PE array.
        th_sb = sb.tile([128, N], MM, tag="th_sb")         # theta (dup)
        ph_sb = sb.tile([128, N], MM, tag="ph_sb")         # phi (dup)
        g_sb = sb.tile([C4, N], F32, tag="g_sb")           # g (pre-transpose)
        gt_sb = sb.tile([128, N_JBLKS * (C4 + 1)], MM, tag="gt_sb")  # [gT|1]
        w1_sb = sb.tile([128, 256], MM, tag="w1_sb")       # dup-M k-tiles
        w2_sb = sb.tile([128, 256], MM, tag="w2_sb")
        w3_sb = sb.tile([128, 128], MM, tag="w3_sb")
        w4_sb = sb.tile([C4, C], MM, tag="w4_sb")
        b123_sb = sb.tile([128, 3], F32, tag="b123_sb")
        b4_sb = sb.tile([128, 2], F32, tag="b4_sb")
        eye_sb = sb.tile([C4, C4], F32, tag="eye_sb")
        ones_sb = sb.tile([1, C4], F32, tag="ones_sb")

        # weights in — direct DMA to f32r tiles (dtype-matched, no cast)
        nc.sync.dma_start(w1_sb[:], aps["w1_d"][:])
        nc.sync.dma_start(w2_sb[:], aps["w2_d"][:])
        nc.sync.dma_start(w3_sb[:], aps["w3_d"][:])
        nc.sync.dma_start(w4_sb[:], aps["w4_d"][:])
        nc.sync.dma_start(b123_sb[:], aps["b123_d"][:])
        nc.sync.dma_start(b4_sb[:], aps["b4_d"][:])
        masks.make_identity(nc, eye_sb[:])
        nc.vector.memset(ones_sb[:], 1.0)
        ones_col = sb.tile([128, N_JBLKS], F32, tag="ones_col")
        nc.vector.memset(ones_col[:], 1.0)
        nc.vector.tensor_copy(
            gt_sb[:].rearrange("p (j c) -> p j c", c=C4 + 1)
            [:, :, C4:C4 + 1],
            ones_col[:].rearrange("p (j c) -> p j c", c=1))

        # x in: f32r copy for matmuls (per n-tile, both k halves so the conv
        # pipeline fills as chunks land) + fp32 copy for the residual adds
        # (only consumed at tails, trickles in the background).
        for n in range(N_ITILES):
            for k in range(2):
                c0 = k * N + n * NI
                nc.sync.dma_start(xr_sb[:, c0:c0 + NI], x_d[:, c0:c0 + NI])
        for c0 in range(0, 2 * N, 1024):
            nc.sync.dma_start(x_sb[:, c0:c0 + 1024],
                              x_d[:, c0:c0 + 1024].bitcast(F32))

        # ---- phase A: theta / phi / g conv1x1; gT via PE transpose ----
        with tc.tile_pool(name="psA", bufs=2, space="PSUM") as psA:

            def conv(dst_sb, w_sb_, bias_col, m, n):
                ps = psA.tile([128, NI], F32, tag="convps")
                for k in range(2):
                    nc.tensor.matmul(
                        ps[0:m, :],
                        w_sb_[:, k * m:(k + 1) * m],
                        xr_sb[:, k * N + n * NI:k * N + (n + 1) * NI],
                        start=(k == 0), stop=(k == 1))
                # bias-add + PSUM->SBUF (+ rounding) in one DVE op
                nc.vector.tensor_scalar_add(
                    dst_sb[0:m, n * NI:(n + 1) * NI], ps[0:m, :],
                    b123_sb[0:m, bias_col:bias_col + 1])

            # same-weight runs: the HAM clock gate only un-throttles on
            # sustained full-activity streams (weight reloads break it)
            for n in range(N_ITILES):
                conv(ph_sb, w2_sb, 1, 128, n)
            for n in range(N_ITILES):
                conv(th_sb, w1_sb, 0, 128, n)
            for n in range(N_ITILES):
                conv(g_sb, w3_sb, 2, C4, n)
                # gT: transpose fresh g 128-col blocks -> [128, 64] each
                for j in range(4 * n, 4 * n + 4):
                    tp = psA.tile([128, C4], F32, tag="tpps")
                    nc.tensor.transpose(tp[:], g_sb[:, j * NJ:(j + 1) * NJ],
                                        eye_sb[:])
                    nc.vector.tensor_copy(
                        gt_sb[:, j * (C4 + 1):j * (C4 + 1) + C4], tp[:])
            # warm burst: re-trigger the clock gate with a dense same-weight
            # matmul run right before the main loop (results unused)
            for r in range(16):
                ps = psA.tile([128, NI], F32, tag="convps",
                              name=f"warm{r}")
                nc.tensor.matmul(ps[:], w2_sb[:, 0:128], xr_sb[:, 0:NI],
                                 start=True, stop=True)

        # ---- main loop ----
        SBANKS = cfg.get("stage_banks", 4)      # psum banks for S^T staging
        HALF = SBANKS // 2 * 512                # cols per staging half
        JPB = HALF // NI                        # j-blocks per exp batch
        ps_stage = st.enter_context(
            tc.tile_pool(name="ps_stage", bufs=2, space="PSUM"))
        ps_pv = st.enter_context(
            tc.tile_pool(name="ps_pv", bufs=2, space="PSUM"))
        ps_misc = st.enter_context(
            tc.tile_pool(name="ps_misc", bufs=1, space="PSUM"))
        pt_pool = st.enter_context(tc.tile_pool(name="pt", bufs=2))
        dv_pool = st.enter_context(tc.tile_pool(name="dv", bufs=2))
        y_pool = st.enter_context(tc.tile_pool(name="yp", bufs=3))

        batches = []
        j = 0
        while j < N_JBLKS:
            nb = min(JPB, N_JBLKS - j)
            batches.append(list(range(j, j + nb)))
            j += nb
        NB = len(batches)

        pvs = [None] * N_ITILES

        def emit_s(i, b):
            # row-packed pairs: even j-blocks on PE rows 0-63, odd on 64-127
            # (theta/phi are duplicated across halves) -> concurrent matmuls
            # and full-array activity for the HAM clock gate.
            stage_t = ps_stage.tile([128, HALF], F32, tag="stage",
                                    name=f"stage_{i}_{b}")
            half = stage_t[:, 0:len(batches[b]) * NI]
            for k, jb in enumerate(batches[b]):
                lo = (k % 2) * C4
                nc.tensor.matmul(
                    half[:, k * NI:(k + 1) * NI],
                    ph_sb[lo:lo + C4, jb * NJ:(jb + 1) * NJ],
                    th_sb[lo:lo + C4, i * NI:(i + 1) * NI],
                    start=True, stop=True)
            return half

        def emit_exp_pv(i, b, half):
            w = len(batches[b]) * NI
            pt = pt_pool.tile([128, HALF], MM, tag="pt")
            nc.scalar.activation(pt[:, 0:w], half[:], Exp)
            pv = pvs[i]
            for k, jb in enumerate(batches[b]):
                nc.tensor.matmul(
                    pv[0:C4 + 1, :],
                    gt_sb[:, jb * (C4 + 1):(jb + 1) * (C4 + 1)],
                    pt[:, k * NI:(k + 1) * NI],
                    start=(jb == 0), stop=(jb == N_JBLKS - 1))

        def emit_tail(i):
            pv = pvs[i]
            lrow = dv_pool.tile([1, NI], F32, tag="lrow")
            nc.vector.tensor_copy(lrow[:], pv[C4:C4 + 1, :])
            recip = dv_pool.tile([1, NI], F32, tag="recip")
            nc.vector.reciprocal_approx_fast(recip[:], lrow[:])
            bc = ps_misc.tile([128, NI], F32, tag="misc")
            nc.tensor.matmul(bc[0:C4, :], ones_sb[:], recip[:],
                             start=True, stop=True)
            bcs = dv_pool.tile([C4, NI], F32, tag="bcs")
            nc.vector.tensor_copy(bcs[:], bc[0:C4, :])
            ag = dv_pool.tile([C4, NI], MM, tag="ag")
            nc.vector.tensor_tensor(ag[:], pv[0:C4, :], bcs[:], Alu.mult)
            for h in range(2):
                z = ps_misc.tile([128, NI], F32, tag="misc")
                nc.tensor.matmul(z[:], w4_sb[:, h * 128:(h + 1) * 128],
                                 ag[:], start=True, stop=True)
                yt = y_pool.tile([128, NI], F32, tag="yt")
                # y = (z + b4) + x
                nc.vector.scalar_tensor_tensor(
                    yt[:], z[:], b4_sb[:, h:h + 1],
                    x_sb[:, h * N + i * NI:h * N + (i + 1) * NI],
                    Alu.add, Alu.add)
                nc.sync.dma_start(
                    y_d[h * 128:(h + 1) * 128, i * NI:(i + 1) * NI], yt[:])

        # software-pipelined emission: S(b+1) lands before exp/PV(b) on the
        # PE stream; the previous i-tile's tail is deferred past the first S
        # batch of the next i-tile so ACT never waits on the tail chain.
        pending_tail = None
        TAIL_AT = 7   # defer prev tail this many batches into the next i-tile
        for i in range(N_ITILES):
            pvs[i] = ps_pv.tile([128, NI], F32, tag="pv", name=f"pv{i}")
            halves = [None] * NB
            halves[0] = emit_s(i, 0)
            for b in range(NB):
                if b + 1 < NB:
                    halves[b + 1] = emit_s(i, b + 1)
                emit_exp_pv(i, b, halves[b])
                if b == TAIL_AT and pending_tail is not None:
                    emit_tail(pending_tail)
                    pending_tail = None
            pending_tail = i
        emit_tail(pending_tail)


def _prepare_core_inputs(x_b, W1, b1, W2, b2, W3, b3, W4, b4):
    def ktile(wT, m):
        # [256, m] -> [128, 2*m] (two k-tiles side by side)
        return np.ascontiguousarray(
            wT.reshape(2, 128, m).transpose(1, 0, 2).reshape(128, 2 * m))

    def dup(wT):
        # duplicate output channels across both halves: [256,64] -> [256,128]
        return np.concatenate([wT, wT], axis=1)

    z64 = np.zeros(C4, np.float32)
    return {
        "x": np.ascontiguousarray(
            x_b.reshape(2, 128, N).transpose(1, 0, 2).reshape(128, 2 * N)),
        "w1t": ktile(dup(W1.T), 128), "w2t": ktile(dup(W2.T), 128),
        "w3t": ktile(W3.T, C4),
        "w4t": np.ascontiguousarray(W4.T),
        "b123": np.ascontiguousarray(
            np.stack([np.r_[b1, b1], np.r_[b2, b2], np.r_[b3, z64]], axis=1)),
        "b4c": np.ascontiguousarray(b4.reshape(2, 128).T),
    }


def kernel(x, W1, b1, W2, b2, W3, b3, W4, b4, _trace=False, _cfg=None):
    from concourse import bass_utils

    cfg = dict(_cfg or {})
    key = tuple(sorted(cfg.items()))
    if key not in _CACHE:
        _CACHE[key] = _build(cfg)
    nc = _CACHE[key]

    x = np.asarray(x, dtype=np.float32)
    xf = x.reshape(B, C, N)
    args = [np.asarray(a, dtype=np.float32)
            for a in (W1, b1, W2, b2, W3, b3, W4, b4)]
    in_maps = [_prepare_core_inputs(xf[b], *args) for b in range(B)]
    res = bass_utils.run_bass_kernel_spmd(
        nc, in_maps, core_ids=list(range(B)), trace=_trace)
    out = np.stack([res.results[b]["y"].reshape(C, H, W) for b in range(B)])
    if _trace:
        kernel.last_exec_time_ns = res.exec_time_ns
    return out
